# revision 29
# baseline (speedup 1.0000x reference)
"""Distributed QK-norm multi-head attention on 8 Trainium2 NeuronCores.

Strategy: tensor-parallel on heads (2 heads/core) through QKV projection and
attention; chunked AllGather of the head-major context (partition-axis
concat); column-parallel output projection. All operands are pre-transposed
and cast to bf16 on host so every matmul is in PE-native layout; f32
accumulation; softmax denominators via a ones-augmented V matmul.

v2 schedule: scalar engine runs ONLY softmax exp (one activation table load);
LN stats via Newton-rsqrt on the vector engine; PSUM evictions on
vector; row-tiled concurrent score matmuls (2 heads in disjoint PE row
groups); batch-1 QKV projection issued after batch-0 attention so it fills
PE gaps under the exp-bound softmax phase; per-chunk AllGather + output
projection drained inside the attention loops.

kernel(**inputs) takes the full unsharded inputs and returns the full
[2, 2048, 1024] float32 output.
"""

from contextlib import ExitStack

import numpy as np

import concourse.bass as bass
import concourse.bacc as bacc
import concourse.tile as tile
import concourse.mybir as mybir

F32 = mybir.dt.float32
BF16 = mybir.dt.bfloat16
I32 = mybir.dt.int32
AF = mybir.ActivationFunctionType
OP = mybir.AluOpType

N_CORES = 8
B, NSEQ, D = 2, 2048, 1024
H, HD = 16, 64
HC = H // N_CORES          # heads per core = 2
T = B * NSEQ               # 4096 tokens
P = 128
NTB = T // P               # 32 token blocks
NTB_B = NSEQ // P          # 16 per batch half
ND = D // P                # 8 contraction tiles
KB = NSEQ // P             # 16 key blocks
QG = 512                   # q-group (moving free dim)
NQG = NSEQ // QG           # 4 q groups
EPS = 1e-5
W = 3 * P + 4              # 388: qkv outputs + 4 mean columns
MAGIC = 0x5F3759DF         # rsqrt bit-trick seed


def build(n_cores: int = N_CORES, trivial_gb: bool = True):
    nc = bacc.Bacc("TRN2", target_bir_lowering=False, debug=False,
                   num_devices=n_cores)

    xT = nc.dram_tensor("xT", [D, T], BF16, kind="ExternalInput")
    wqkvT = nc.dram_tensor("wqkvT", [D, W], BF16, kind="ExternalInput")
    bqkv = nc.dram_tensor("bqkv", [1, W], BF16, kind="ExternalInput")
    wpT = nc.dram_tensor("wpT", [D, P], BF16, kind="ExternalInput")
    bp = nc.dram_tensor("bp", [P, 1], F32, kind="ExternalInput")
    qg2 = nc.dram_tensor("qg2", [P, 1], F32, kind="ExternalInput")
    qb2 = nc.dram_tensor("qb2", [P, 1], F32, kind="ExternalInput")
    kg2 = nc.dram_tensor("kg2", [P, 1], F32, kind="ExternalInput")
    kb2 = nc.dram_tensor("kb2", [P, 1], F32, kind="ExternalInput")
    ident = nc.dram_tensor("ident", [P, P], BF16, kind="ExternalInput")
    outT = nc.dram_tensor("outT", [P, T], F32, kind="ExternalOutput")

    with tile.TileContext(nc) as tc, ExitStack() as ctx:
        pools = {}
        for name, bufs, space in [
            ("xt", 1, "SBUF"), ("wq", 1, "SBUF"), ("wp", 1, "SBUF"),
            ("const", 1, "SBUF"), ("qkt", 1, "SBUF"), ("vp", 1, "SBUF"),
            ("raw", 1, "SBUF"), ("stat", 1, "SBUF"), ("sq", 2, "SBUF"),
            ("tok", 4, "SBUF"), ("at", 3, "SBUF"), ("rb", 3, "SBUF"),
            ("cstage", 1, "SBUF"), ("pr", 6, "SBUF"),
            ("osb", 2, "SBUF"), ("dram", 1, "DRAM"),
            ("ps_s", 2, "PSUM"), ("ps_ctx", 2, "PSUM"), ("ps_misc", 2, "PSUM"),
        ]:
            pools[name] = ctx.enter_context(
                tc.tile_pool(name=name, bufs=bufs, space=space))

        # ---- constants first: tiny DMAs must not queue behind bulk loads ----
        cp = pools["const"]
        bqkv_sb = cp.tile([1, W], BF16, name="bqkv_sb")
        nc.sync.dma_start(bqkv_sb[:], bqkv[:])
        bp_sb = cp.tile([P, 1], F32, name="bp_sb")
        nc.sync.dma_start(bp_sb[:], bp[:])
        gb_sb = {}
        for nm, src in (("qg2", qg2), ("qb2", qb2), ("kg2", kg2), ("kb2", kb2)):
            t_ = cp.tile([P, 1], F32, name=f"{nm}_sb")
            nc.sync.dma_start(t_[:], src[:])
            gb_sb[nm] = t_
        ident_sb = cp.tile([P, P], BF16, name="ident_sb")
        nc.sync.dma_start(ident_sb[:], ident[:])
        ones_sb = cp.tile([1, P], BF16, name="ones_sb")
        nc.vector.memset(ones_sb[:], 1.0)
        zero_sb = cp.tile([P, 1], F32, name="zero_sb")
        nc.vector.memset(zero_sb[:], 0.0)

        # ---- weights + x: interleave so chunk 0 + wq land together ----
        wq_sb = [pools["wq"].tile([P, W], BF16, name=f"wq{dt}")
                 for dt in range(ND)]
        wp_sb = [pools["wp"].tile([P, P], BF16, name=f"wp{dt}")
                 for dt in range(ND)]
        xt_sb = [pools["xt"].tile([P, T], BF16, name=f"xt{dt}")
                 for dt in range(ND)]

        def load_xt_cols(c0, c1):
            for dt in range(ND):
                nc.sync.dma_start(
                    xt_sb[dt][:, c0 * P:c1 * P],
                    xT[dt * P:(dt + 1) * P, c0 * P:c1 * P])

        def load_xt_chunk(ch):
            load_xt_cols(ch * (QG // P), (ch + 1) * (QG // P))

        for dt in range(ND):
            nc.sync.dma_start(wq_sb[dt][:], wqkvT[dt * P:(dt + 1) * P, :])
            nc.sync.dma_start(xt_sb[dt][:, 0:QG], xT[dt * P:(dt + 1) * P, 0:QG])
        load_xt_chunk(1)

        # warm the exp activation table before phase C needs it
        wex_in = cp.tile([1, 8], F32, name="wex_in")
        nc.vector.memset(wex_in[:], 0.0)
        wex_out = cp.tile([1, 8], BF16, name="wex_out")
        nc.scalar.activation(wex_out[:], wex_in[:], AF.Exp,
                             bias=zero_sb[0:1, :], scale=1.0)

        # persistent tensors
        qt_sb = [pools["qkt"].tile([P, NSEQ], BF16, name=f"qt{b}") for b in range(B)]
        kt_sb = [pools["qkt"].tile([P, NSEQ], BF16, name=f"kt{b}") for b in range(B)]
        vp_sb = [pools["vp"].tile([P, KB, HC * (HD + 1)], BF16, name=f"vp{b}")
                 for b in range(B)]
        for b in range(B):
            for h in range(HC):
                nc.vector.memset(
                    vp_sb[b][:, :, h * (HD + 1) + HD:h * (HD + 1) + HD + 1], 1.0)
        qkraw = [pools["raw"].tile([P, NTB_B, 2 * P], BF16, name=f"qkraw{b}")
                 for b in range(B)]
        svar = [pools["stat"].tile([P, 4 * NTB_B], F32, name=f"svar{b}")
                for b in range(B)]
        smu = [pools["stat"].tile([P, 4 * NTB_B], F32, name=f"smu{b}")
               for b in range(B)]
        rstd_all = [pools["stat"].tile([P, 4 * NTB_B], F32, name=f"rstd{b}")
                    for b in range(B)]
        cstage = [pools["cstage"].tile([P, NSEQ], BF16, name=f"cstage{b}")
                  for b in range(B)]

        warm_in = pools["dram"].tile([P, 4], BF16, name="warm_in")
        warm_out = pools["dram"].tile([P * n_cores, 4], BF16, name="warm_out",
                                      addr_space="Shared")
        warm_sb = cp.tile([P, 4], BF16, name="warm_sb")
        nc.vector.memset(warm_sb[:], 0.0)
        nc.sync.dma_start(warm_in[:], warm_sb[:])
        nc.gpsimd.collective_compute(
            "AllGather", OP.bypass, replica_groups=[list(range(n_cores))],
            ins=[warm_in[:].opt()], outs=[warm_out[:].opt()])

        bounce = [[pools["dram"].tile([P, QG], BF16, name=f"bounce{b}_{qg}")
                   for qg in range(NQG)] for b in range(B)]
        gathered = [[pools["dram"].tile([P * n_cores, QG], BF16,
                                        name=f"gath{b}_{qg}",
                                        addr_space="Shared")
                     for qg in range(NQG)] for b in range(B)]
        # final chunk is gathered in two halves to shorten the drain tail
        HQG = QG // 2
        bounce_h = [pools["dram"].tile([P, HQG], BF16, name=f"bounce_h{i}")
                    for i in range(2)]
        gathered_h = [pools["dram"].tile([P * n_cores, HQG], BF16,
                                         name=f"gath_h{i}",
                                         addr_space="Shared")
                      for i in range(2)]

        # ---- phase A: QKV projection for one token block ----
        # Batch 0 evicts on the (then idle) scalar engine; batch 1 runs under
        # phase C(0) where scalar is exp-saturated, so it evicts on vector.
        def a_block(b, tb_b, scalar_evict=False):
            tb = b * NTB_B + tb_b
            psp = pools["ps_misc"].tile([P, W], F32, name="psp", tag="misc")
            for dt in range(ND):
                nc.tensor.matmul(psp[:], xt_sb[dt][:, tb * P:(tb + 1) * P],
                                 wq_sb[dt][:], start=(dt == 0), stop=False)
            nc.tensor.matmul(psp[:], ones_sb[:], bqkv_sb[:],
                             start=False, stop=True)
            if scalar_evict:
                # scalar is idle pre-softmax; split eviction across engines
                nc.scalar.copy(qkraw[b][:, tb_b, :], psp[:, 0:2 * P])
                nc.scalar.copy(
                    vp_sb[b][:, tb_b, :].rearrange("p (h w) -> p h w",
                                                   h=HC)[:, :, 0:HD],
                    psp[:, 2 * P:3 * P].rearrange("p (h w) -> p h w", h=HC))
                nc.vector.tensor_copy(smu[b][:, 4 * tb_b:4 * tb_b + 4],
                                      psp[:, 3 * P:W])
            else:
                nc.vector.tensor_copy(qkraw[b][:, tb_b, :], psp[:, 0:2 * P])
                nc.vector.tensor_copy(
                    vp_sb[b][:, tb_b, :].rearrange("p (h w) -> p h w",
                                                   h=HC)[:, :, 0:HD],
                    psp[:, 2 * P:3 * P].rearrange("p (h w) -> p h w", h=HC))
                nc.vector.tensor_copy(smu[b][:, 4 * tb_b:4 * tb_b + 4],
                                      psp[:, 3 * P:W])
            sq = pools["sq"].tile([P, 2 * P], F32, name="sq", tag="sq")
            nc.vector.tensor_tensor(out=sq[:], in0=qkraw[b][:, tb_b, :],
                                    in1=qkraw[b][:, tb_b, :], op=OP.mult)
            nc.vector.tensor_reduce(
                svar[b][:, 4 * tb_b:4 * tb_b + 4],
                sq[:].rearrange("p (g w) -> p g w", g=4),
                axis=mybir.AxisListType.X, op=OP.add)

        # ---- stats: Newton-rsqrt of var+eps on the vector engine ----
        def stats(b, half):
            lo, hi = 32 * half, 32 * (half + 1)
            n = hi - lo
            musq = pools["sq"].tile([P, n], F32, name="musq", tag="musq")
            nc.vector.tensor_tensor(out=musq[:], in0=smu[b][:, lo:hi],
                                    in1=smu[b][:, lo:hi], op=OP.mult)
            varb = pools["sq"].tile([P, n], F32, name="varb", tag="varb")
            nc.vector.scalar_tensor_tensor(
                out=varb[:], in0=svar[b][:, lo:hi], scalar=1.0 / HD,
                in1=musq[:], op0=OP.mult, op1=OP.subtract)
            nc.vector.tensor_scalar(varb[:], varb[:], EPS, None, op0=OP.add)
            ib = pools["sq"].tile([P, n], I32, name="ib", tag="ib")
            nc.vector.tensor_scalar(ib[:], varb[:].bitcast(I32), 1, None,
                                    op0=OP.logical_shift_right)
            nc.vector.tensor_scalar(ib[:], ib[:], -1, None, op0=OP.bitwise_xor)
            nc.vector.tensor_scalar(ib[:], ib[:], MAGIC + 1, None, op0=OP.add)
            y = pools["sq"].tile([P, n], F32, name="y", tag="y")
            nc.vector.tensor_copy(y[:], ib[:].bitcast(F32))
            t_ = pools["sq"].tile([P, n], F32, name="t", tag="t")
            for it in range(2):
                nc.vector.tensor_tensor(out=t_[:], in0=y[:], in1=y[:],
                                        op=OP.mult)
                nc.vector.tensor_tensor(out=t_[:], in0=t_[:], in1=varb[:],
                                        op=OP.mult)
                nc.vector.tensor_scalar(t_[:], t_[:], -0.5, 1.5,
                                        op0=OP.mult, op1=OP.add)
                dst = rstd_all[b][:, lo:hi] if it == 1 else y[:]
                nc.vector.tensor_tensor(out=dst, in0=y[:], in1=t_[:],
                                        op=OP.mult)

        # ---- phase B: per-head QK layernorm + transpose to [hd, tokens] ----
        def b_block(b, tb_b, scalar_evict=False):
            for qk, (g2, b2, dst) in enumerate((
                    (gb_sb["qg2"], gb_sb["qb2"], qt_sb[b]),
                    (gb_sb["kg2"], gb_sb["kb2"], kt_sb[b]))):
                tokt = pools["tok"].tile([P, P], BF16, name="tokt", tag="tok")
                for h in range(HC):
                    i = 4 * tb_b + 2 * qk + h
                    nc.vector.tensor_scalar(
                        tokt[:, h * HD:(h + 1) * HD],
                        qkraw[b][:, tb_b,
                                 qk * P + h * HD:qk * P + (h + 1) * HD],
                        smu[b][:, i:i + 1], rstd_all[b][:, i:i + 1],
                        op0=OP.subtract, op1=OP.mult)
                # hardware XBAR transpose on the DMA path: SBUF->SBUF,
                # ~14ns per 16x128 tile -- frees the PE and the PSUM ring
                dslice = dst[:, tb_b * P:(tb_b + 1) * P]
                nc.sync.dma_start_transpose(dslice, tokt[:])
                if not trivial_gb:
                    nc.vector.tensor_scalar(dslice, dslice, g2[:], b2[:],
                                            op0=OP.mult, op1=OP.add)

        # ---- phase C: scores (row-tiled) -> exp -> ctx, then normalize ----
        def scores(b, qg, kb, pss):
            for h in range(HC):
                nc.tensor.matmul(
                    pss[:, h * QG:(h + 1) * QG],
                    kt_sb[b][h * HD:(h + 1) * HD, kb * P:(kb + 1) * P],
                    qt_sb[b][h * HD:(h + 1) * HD, qg * QG:(qg + 1) * QG],
                    start=True, stop=True)

        def c_qg(b, qg, fillers=()):
            fillers = list(fillers)
            ctx_ps = [pools["ps_ctx"].tile([HD + 1, QG], F32, name="ctx",
                                           tag="ctx") for _ in range(HC)]
            pss_tiles = {}
            pss_tiles[0] = pools["ps_s"].tile([P, 2 * QG], F32, name="pss",
                                              tag="pss")
            scores(b, qg, 0, pss_tiles[0])
            nfill = len(fillers)
            fill_at = {(i + 1) * KB // (nfill + 1): i for i in range(nfill)}
            for kb in range(KB):
                if kb + 1 < KB:
                    pss_tiles[kb + 1] = pools["ps_s"].tile(
                        [P, 2 * QG], F32, name="pss", tag="pss")
                    scores(b, qg, kb + 1, pss_tiles[kb + 1])
                at = pools["at"].tile([P, 2 * QG], BF16, name="at", tag="at")
                nc.scalar.activation(at[:], pss_tiles.pop(kb)[:], AF.Exp,
                                     bias=zero_sb[:], scale=0.125)
                for h in range(HC):
                    nc.tensor.matmul(
                        ctx_ps[h][:],
                        vp_sb[b][:, kb, h * (HD + 1):(h + 1) * (HD + 1)],
                        at[:, h * QG:(h + 1) * QG],
                        start=(kb == 0), stop=(kb == KB - 1))
                if kb in fill_at:
                    fillers[fill_at[kb]]()
            for h in range(HC):
                den = pools["rb"].tile([1, QG], F32, name="den", tag="den")
                nc.vector.tensor_copy(den[:], ctx_ps[h][HD:HD + 1, :])
                rec = pools["rb"].tile([1, QG], F32, name="rec", tag="rec")
                nc.vector.reciprocal_approx_fast(out=rec[:], in_=den[:])
                rb = pools["rb"].tile([HD, QG], F32, name="rb", tag="rb")
                nc.gpsimd.partition_broadcast(rb[:], rec[:])
                nc.vector.tensor_tensor(
                    out=cstage[b][h * HD:(h + 1) * HD,
                                  qg * QG:(qg + 1) * QG],
                    in0=ctx_ps[h][0:HD, :], in1=rb[:], op=OP.mult)
            nc.sync.dma_start(bounce[b][qg][:],
                              cstage[b][:, qg * QG:(qg + 1) * QG])
            nc.gpsimd.collective_compute(
                "AllGather", OP.bypass,
                replica_groups=[list(range(n_cores))],
                ins=[bounce[b][qg][:].opt()],
                outs=[gathered[b][qg][:].opt()])

        # ---- phase D: output projection of one gathered chunk ----
        def d_drain(b, qg, half=None):
            cols = QG if half is None else HQG
            src = gathered[b][qg] if half is None else gathered_h[half]
            col0 = b * NSEQ + qg * QG + (0 if half is None else half * HQG)
            pso = pools["ps_misc"].tile([P, cols], F32, name="pso", tag="misc")
            for dt in range(n_cores):
                prt = pools["pr"].tile([P, cols], BF16, name="prt", tag="pr")
                nc.gpsimd.dma_start(prt[:], src[dt * P:(dt + 1) * P, :])
                nc.tensor.matmul(pso[:], wp_sb[dt][:], prt[:],
                                 start=(dt == 0), stop=(dt == n_cores - 1))
            osb = pools["osb"].tile([P, cols], F32, name="osb", tag="osb")
            nc.vector.tensor_scalar(osb[:], pso[:], bp_sb[:], None, op0=OP.add)
            nc.sync.dma_start(outT[:, col0:col0 + cols], osb[:])

        # ---- schedule ----
        # Batch-0 first half evicts on the idle scalar engine; C(0) is issued
        # as soon as 4 token blocks are normalized, with the rest of B(0),
        # then all of A(1)/B(1), woven in as PE filler. Output drains only
        # cover gathers that are >= 2 chunks old so they never stall the PE
        # stream on an in-flight collective.
        for tb_b in range(NTB_B):
            if tb_b % 4 == 0 and tb_b // 4 + 2 < 4:
                load_xt_chunk(tb_b // 4 + 2)
            a_block(0, tb_b, scalar_evict=(tb_b < 8))
            if tb_b == NTB_B // 2 - 1:
                stats(0, 0)
        stats(0, 1)
        for tb_b in range(NTB_B):
            b_block(0, tb_b)
        load_xt_chunk(4)
        load_xt_chunk(5)
        for dt in range(ND):
            nc.sync.dma_start(wp_sb[dt][:], wpT[dt * P:(dt + 1) * P, :])

        def a1_filler(tb_b):
            def f():
                if tb_b % 4 == 0 and tb_b // 4 + 6 < T // QG:
                    load_xt_chunk(tb_b // 4 + 6)
                a_block(1, tb_b)
                if tb_b == NTB_B // 2 - 1:
                    stats(1, 0)
                elif tb_b == NTB_B - 1:
                    stats(1, 1)
            return f

        c_qg(0, 0, fillers=[a1_filler(j) for j in range(4)])
        c_qg(0, 1, fillers=[a1_filler(4 + j) for j in range(4)])
        c_qg(0, 2, fillers=[a1_filler(8 + j) for j in range(4)])
        c_qg(0, 3, fillers=[a1_filler(12 + j) for j in range(4)])
        for tb_b in range(NTB_B):
            b_block(1, tb_b)

        c_qg(1, 0)
        c_qg(1, 1, fillers=[lambda: d_drain(0, 0)])
        c_qg(1, 2, fillers=[lambda: d_drain(0, 1), lambda: d_drain(1, 0)])
        c_qg(1, 3, fillers=[lambda: d_drain(0, 2), lambda: d_drain(0, 3),
                            lambda: d_drain(1, 1)])
        d_drain(1, 2)
        d_drain(1, 3)

    nc.compile()
    return nc


def prep_inputs(inputs):
    """Host-side prep: slice/transpose/cast per core. Returns (in_maps, trivial_gb)."""
    import ml_dtypes
    bf16 = ml_dtypes.bfloat16

    q = np.asarray(inputs["query"], np.float32)
    Wq, Wk, Wv, Wp = (np.asarray(inputs[k], np.float32)
                      for k in ("Wq", "Wk", "Wv", "Wp"))
    bq, bk, bv, bpv = (np.asarray(inputs[k], np.float32)
                       for k in ("bq", "bk", "bv", "bp"))
    qg, qb, kg, kb = (np.asarray(inputs[k], np.float32)
                      for k in ("q_gamma", "q_beta", "k_gamma", "k_beta"))

    trivial_gb = bool(
        np.all(qg == 1.0) and np.all(kg == 1.0)
        and np.all(qb == 0.0) and np.all(kb == 0.0))

    xT = np.ascontiguousarray(q.reshape(T, D).T).astype(bf16)
    identity = np.eye(P, dtype=bf16)
    in_maps = []
    for c in range(N_CORES):
        sl = slice(c * P, (c + 1) * P)
        wq_c, wk_c, wv_c = Wq[sl].T, Wk[sl].T, Wv[sl].T  # [1024, 128] each
        mean_cols = np.stack([
            wq_c[:, 0:HD].mean(axis=1), wq_c[:, HD:2 * HD].mean(axis=1),
            wk_c[:, 0:HD].mean(axis=1), wk_c[:, HD:2 * HD].mean(axis=1),
        ], axis=1)                                        # [1024, 4]
        wqkvT = np.concatenate([wq_c, wk_c, wv_c, mean_cols],
                               axis=1).astype(bf16)       # [1024, 388]
        bq_c, bk_c, bv_c = bq[sl], bk[sl], bv[sl]
        bias_means = np.array([
            bq_c[0:HD].mean(), bq_c[HD:].mean(),
            bk_c[0:HD].mean(), bk_c[HD:].mean()], np.float32)
        bqkv = np.concatenate([bq_c, bk_c, bv_c, bias_means])[None, :].astype(bf16)
        in_maps.append({
            "xT": xT,
            "wqkvT": np.ascontiguousarray(wqkvT),
            "bqkv": np.ascontiguousarray(bqkv),
            "wpT": np.ascontiguousarray(Wp[sl].T).astype(bf16),
            "bp": np.ascontiguousarray(bpv[sl].reshape(P, 1)),
            "qg2": np.tile(qg, HC).reshape(P, 1).astype(np.float32),
            "qb2": np.tile(qb, HC).reshape(P, 1).astype(np.float32),
            "kg2": np.tile(kg, HC).reshape(P, 1).astype(np.float32),
            "kb2": np.tile(kb, HC).reshape(P, 1).astype(np.float32),
            "ident": identity,
        })
    return in_maps, trivial_gb


def assemble_output(results):
    outT = np.concatenate([np.asarray(r["outT"], np.float32) for r in results],
                          axis=0)           # [1024, 4096]
    return np.ascontiguousarray(outT.T).reshape(B, NSEQ, D)


_CACHE = {}


def kernel(**inputs):
    from concourse.bass_utils import run_bass_kernel_spmd

    in_maps, _trivial = prep_inputs(inputs)
    key = ("nc", _trivial)
    if key not in _CACHE:
        _CACHE[key] = build(trivial_gb=_trivial)
    nc = _CACHE[key]
    res = run_bass_kernel_spmd(nc, in_maps, core_ids=list(range(N_CORES)))
    return assemble_output(res.results)


# revision 31
# speedup vs baseline: 1.1979x; 1.1979x over previous
"""Distributed QK-norm multi-head attention on 8 Trainium2 NeuronCores.

Strategy: tensor-parallel on heads (2 heads/core) through QKV projection and
attention; chunked AllGather of the head-major context (partition-axis
concat); column-parallel output projection. All operands are pre-transposed
and cast to bf16 on host so every matmul is in PE-native layout; f32
accumulation; softmax denominators via a ones-augmented V matmul.

v2 schedule: scalar engine runs ONLY softmax exp (one activation table load);
LN stats via Newton-rsqrt on the vector engine; PSUM evictions on
vector; row-tiled concurrent score matmuls (2 heads in disjoint PE row
groups); batch-1 QKV projection issued after batch-0 attention so it fills
PE gaps under the exp-bound softmax phase; per-chunk AllGather + output
projection drained inside the attention loops.

kernel(**inputs) takes the full unsharded inputs and returns the full
[2, 2048, 1024] float32 output.
"""

from contextlib import ExitStack

import numpy as np

import concourse.bass as bass
import concourse.bacc as bacc
import concourse.tile as tile
import concourse.mybir as mybir

F32 = mybir.dt.float32
BF16 = mybir.dt.bfloat16
I32 = mybir.dt.int32
AF = mybir.ActivationFunctionType
OP = mybir.AluOpType

N_CORES = 8
B, NSEQ, D = 2, 2048, 1024
H, HD = 16, 64
HC = H // N_CORES          # heads per core = 2
T = B * NSEQ               # 4096 tokens
P = 128
NTB = T // P               # 32 token blocks
NTB_B = NSEQ // P          # 16 per batch half
ND = D // P                # 8 contraction tiles
KB = NSEQ // P             # 16 key blocks
QG = 512                   # q-group (moving free dim)
NQG = NSEQ // QG           # 4 q groups
EPS = 1e-5
W = 3 * P + 4              # 388: qkv outputs + 4 mean columns
MAGIC = 0x5F3759DF         # rsqrt bit-trick seed


def build(n_cores: int = N_CORES, trivial_gb: bool = True):
    nc = bacc.Bacc("TRN2", target_bir_lowering=False, debug=False,
                   num_devices=n_cores)

    xT = nc.dram_tensor("xT", [D, T], BF16, kind="ExternalInput")
    wqkvT = nc.dram_tensor("wqkvT", [D, W], BF16, kind="ExternalInput")
    bqkv = nc.dram_tensor("bqkv", [1, W], BF16, kind="ExternalInput")
    wpT = nc.dram_tensor("wpT", [D, P], BF16, kind="ExternalInput")
    bp = nc.dram_tensor("bp", [P, 1], F32, kind="ExternalInput")
    qg2 = nc.dram_tensor("qg2", [P, 1], F32, kind="ExternalInput")
    qb2 = nc.dram_tensor("qb2", [P, 1], F32, kind="ExternalInput")
    kg2 = nc.dram_tensor("kg2", [P, 1], F32, kind="ExternalInput")
    kb2 = nc.dram_tensor("kb2", [P, 1], F32, kind="ExternalInput")
    ident = nc.dram_tensor("ident", [P, P], BF16, kind="ExternalInput")
    outT = nc.dram_tensor("outT", [P, T], F32, kind="ExternalOutput")

    with tile.TileContext(nc) as tc, ExitStack() as ctx:
        pools = {}
        for name, bufs, space in [
            ("xt", 1, "SBUF"), ("wq", 1, "SBUF"), ("wp", 1, "SBUF"),
            ("const", 1, "SBUF"), ("qkt", 1, "SBUF"), ("vp", 1, "SBUF"),
            ("raw", 1, "SBUF"), ("stat", 1, "SBUF"), ("sq", 2, "SBUF"),
            ("tok", 4, "SBUF"), ("at", 3, "SBUF"), ("rb", 3, "SBUF"),
            ("cstage", 1, "SBUF"), ("pr", 6, "SBUF"),
            ("osb", 2, "SBUF"), ("dram", 1, "DRAM"),
            ("ps_s", 2, "PSUM"), ("ps_ctx", 2, "PSUM"), ("ps_misc", 2, "PSUM"),
        ]:
            pools[name] = ctx.enter_context(
                tc.tile_pool(name=name, bufs=bufs, space=space))

        # ---- constants first: tiny DMAs must not queue behind bulk loads ----
        cp = pools["const"]
        bqkv_sb = cp.tile([1, W], BF16, name="bqkv_sb")
        nc.sync.dma_start(bqkv_sb[:], bqkv[:])
        bp_sb = cp.tile([P, 1], F32, name="bp_sb")
        nc.sync.dma_start(bp_sb[:], bp[:])
        gb_sb = {}
        for nm, src in (("qg2", qg2), ("qb2", qb2), ("kg2", kg2), ("kb2", kb2)):
            t_ = cp.tile([P, 1], F32, name=f"{nm}_sb")
            nc.sync.dma_start(t_[:], src[:])
            gb_sb[nm] = t_
        ident_sb = cp.tile([P, P], BF16, name="ident_sb")
        nc.sync.dma_start(ident_sb[:], ident[:])
        ones_sb = cp.tile([1, P], BF16, name="ones_sb")
        nc.vector.memset(ones_sb[:], 1.0)
        zero_sb = cp.tile([P, 1], F32, name="zero_sb")
        nc.vector.memset(zero_sb[:], 0.0)

        # ---- weights + x: interleave so chunk 0 + wq land together ----
        wq_sb = [pools["wq"].tile([P, W], BF16, name=f"wq{dt}")
                 for dt in range(ND)]
        wp_sb = [pools["wp"].tile([P, P], BF16, name=f"wp{dt}")
                 for dt in range(ND)]
        xt_sb = [pools["xt"].tile([P, T], BF16, name=f"xt{dt}")
                 for dt in range(ND)]

        def load_xt_cols(c0, c1):
            for dt in range(ND):
                nc.sync.dma_start(
                    xt_sb[dt][:, c0 * P:c1 * P],
                    xT[dt * P:(dt + 1) * P, c0 * P:c1 * P])

        def load_xt_chunk(ch):
            load_xt_cols(ch * (QG // P), (ch + 1) * (QG // P))

        for dt in range(ND):
            nc.sync.dma_start(wq_sb[dt][:], wqkvT[dt * P:(dt + 1) * P, :])
            nc.sync.dma_start(xt_sb[dt][:, 0:QG], xT[dt * P:(dt + 1) * P, 0:QG])
        load_xt_chunk(1)

        # warm the exp activation table before phase C needs it
        wex_in = cp.tile([1, 8], F32, name="wex_in")
        nc.vector.memset(wex_in[:], 0.0)
        wex_out = cp.tile([1, 8], BF16, name="wex_out")
        nc.scalar.activation(wex_out[:], wex_in[:], AF.Exp,
                             bias=zero_sb[0:1, :], scale=1.0)

        # persistent tensors
        qt_sb = [pools["qkt"].tile([P, NSEQ], BF16, name=f"qt{b}") for b in range(B)]
        kt_sb = [pools["qkt"].tile([P, NSEQ], BF16, name=f"kt{b}") for b in range(B)]
        vp_sb = [pools["vp"].tile([P, KB, HC * (HD + 1)], BF16, name=f"vp{b}")
                 for b in range(B)]
        for b in range(B):
            for h in range(HC):
                nc.vector.memset(
                    vp_sb[b][:, :, h * (HD + 1) + HD:h * (HD + 1) + HD + 1], 1.0)
        qkraw = [pools["raw"].tile([P, NTB_B, 2 * P], BF16, name=f"qkraw{b}")
                 for b in range(B)]
        svar = [pools["stat"].tile([P, 4 * NTB_B], F32, name=f"svar{b}")
                for b in range(B)]
        smu = [pools["stat"].tile([P, 4 * NTB_B], F32, name=f"smu{b}")
               for b in range(B)]
        rstd_all = [pools["stat"].tile([P, 4 * NTB_B], F32, name=f"rstd{b}")
                    for b in range(B)]
        cstage = [pools["cstage"].tile([P, NSEQ], BF16, name=f"cstage{b}")
                  for b in range(B)]

        warm_in = pools["dram"].tile([P, 4], BF16, name="warm_in")
        warm_out = pools["dram"].tile([P * n_cores, 4], BF16, name="warm_out",
                                      addr_space="Shared")
        warm_sb = cp.tile([P, 4], BF16, name="warm_sb")
        nc.vector.memset(warm_sb[:], 0.0)
        nc.sync.dma_start(warm_in[:], warm_sb[:])
        nc.gpsimd.collective_compute(
            "AllGather", OP.bypass, replica_groups=[list(range(n_cores))],
            ins=[warm_in[:].opt()], outs=[warm_out[:].opt()])

        bounce = [[pools["dram"].tile([P, QG], BF16, name=f"bounce{b}_{qg}")
                   for qg in range(NQG)] for b in range(B)]
        gathered = [[pools["dram"].tile([P * n_cores, QG], BF16,
                                        name=f"gath{b}_{qg}",
                                        addr_space="Shared")
                     for qg in range(NQG)] for b in range(B)]
        # final chunk is gathered in two halves to shorten the drain tail
        HQG = QG // 2
        bounce_h = [pools["dram"].tile([P, HQG], BF16, name=f"bounce_h{i}")
                    for i in range(2)]
        gathered_h = [pools["dram"].tile([P * n_cores, HQG], BF16,
                                         name=f"gath_h{i}",
                                         addr_space="Shared")
                      for i in range(2)]

        # ---- phase A: QKV projection for one token block ----
        # Batch 0 evicts on the (then idle) scalar engine; batch 1 runs under
        # phase C(0) where scalar is exp-saturated, so it evicts on vector.
        def a_block(b, tb_b, scalar_evict=False):
            tb = b * NTB_B + tb_b
            psp = pools["ps_misc"].tile([P, W], F32, name="psp", tag="misc")
            for dt in range(ND):
                nc.tensor.matmul(psp[:], xt_sb[dt][:, tb * P:(tb + 1) * P],
                                 wq_sb[dt][:], start=(dt == 0), stop=False)
            nc.tensor.matmul(psp[:], ones_sb[:], bqkv_sb[:],
                             start=False, stop=True)
            if scalar_evict:
                # scalar is idle pre-softmax; split eviction across engines
                nc.scalar.copy(qkraw[b][:, tb_b, :], psp[:, 0:2 * P])
                nc.scalar.copy(
                    vp_sb[b][:, tb_b, :].rearrange("p (h w) -> p h w",
                                                   h=HC)[:, :, 0:HD],
                    psp[:, 2 * P:3 * P].rearrange("p (h w) -> p h w", h=HC))
                nc.vector.tensor_copy(smu[b][:, 4 * tb_b:4 * tb_b + 4],
                                      psp[:, 3 * P:W])
            else:
                nc.vector.tensor_copy(qkraw[b][:, tb_b, :], psp[:, 0:2 * P])
                nc.vector.tensor_copy(
                    vp_sb[b][:, tb_b, :].rearrange("p (h w) -> p h w",
                                                   h=HC)[:, :, 0:HD],
                    psp[:, 2 * P:3 * P].rearrange("p (h w) -> p h w", h=HC))
                nc.vector.tensor_copy(smu[b][:, 4 * tb_b:4 * tb_b + 4],
                                      psp[:, 3 * P:W])
            sq = pools["sq"].tile([P, 2 * P], F32, name="sq", tag="sq")
            nc.vector.tensor_tensor(out=sq[:], in0=qkraw[b][:, tb_b, :],
                                    in1=qkraw[b][:, tb_b, :], op=OP.mult)
            nc.vector.tensor_reduce(
                svar[b][:, 4 * tb_b:4 * tb_b + 4],
                sq[:].rearrange("p (g w) -> p g w", g=4),
                axis=mybir.AxisListType.X, op=OP.add)

        # ---- stats: Newton-rsqrt of var+eps on the vector engine ----
        def stats(b, half):
            lo, hi = 32 * half, 32 * (half + 1)
            n = hi - lo
            musq = pools["sq"].tile([P, n], F32, name="musq", tag="musq")
            nc.vector.tensor_tensor(out=musq[:], in0=smu[b][:, lo:hi],
                                    in1=smu[b][:, lo:hi], op=OP.mult)
            varb = pools["sq"].tile([P, n], F32, name="varb", tag="varb")
            nc.vector.scalar_tensor_tensor(
                out=varb[:], in0=svar[b][:, lo:hi], scalar=1.0 / HD,
                in1=musq[:], op0=OP.mult, op1=OP.subtract)
            nc.vector.tensor_scalar(varb[:], varb[:], EPS, None, op0=OP.add)
            ib = pools["sq"].tile([P, n], I32, name="ib", tag="ib")
            nc.vector.tensor_scalar(ib[:], varb[:].bitcast(I32), 1, None,
                                    op0=OP.logical_shift_right)
            nc.vector.tensor_scalar(ib[:], ib[:], -1, None, op0=OP.bitwise_xor)
            nc.vector.tensor_scalar(ib[:], ib[:], MAGIC + 1, None, op0=OP.add)
            y = pools["sq"].tile([P, n], F32, name="y", tag="y")
            nc.vector.tensor_copy(y[:], ib[:].bitcast(F32))
            t_ = pools["sq"].tile([P, n], F32, name="t", tag="t")
            for it in range(2):
                nc.vector.tensor_tensor(out=t_[:], in0=y[:], in1=y[:],
                                        op=OP.mult)
                nc.vector.tensor_tensor(out=t_[:], in0=t_[:], in1=varb[:],
                                        op=OP.mult)
                nc.vector.tensor_scalar(t_[:], t_[:], -0.5, 1.5,
                                        op0=OP.mult, op1=OP.add)
                dst = rstd_all[b][:, lo:hi] if it == 1 else y[:]
                nc.vector.tensor_tensor(out=dst, in0=y[:], in1=t_[:],
                                        op=OP.mult)

        # ---- phase B: per-head QK layernorm + transpose to [hd, tokens] ----
        def b_block(b, tb_b, scalar_evict=False):
            for qk, (g2, b2, dst) in enumerate((
                    (gb_sb["qg2"], gb_sb["qb2"], qt_sb[b]),
                    (gb_sb["kg2"], gb_sb["kb2"], kt_sb[b]))):
                tokt = pools["tok"].tile([P, P], BF16, name="tokt", tag="tok")
                for h in range(HC):
                    i = 4 * tb_b + 2 * qk + h
                    nc.vector.tensor_scalar(
                        tokt[:, h * HD:(h + 1) * HD],
                        qkraw[b][:, tb_b,
                                 qk * P + h * HD:qk * P + (h + 1) * HD],
                        smu[b][:, i:i + 1], rstd_all[b][:, i:i + 1],
                        op0=OP.subtract, op1=OP.mult)
                pst = pools["ps_misc"].tile([P, P], BF16, name="pst",
                                            tag="misc")
                nc.tensor.transpose(pst[:], tokt[:], ident_sb[:])
                if scalar_evict and trivial_gb:
                    nc.scalar.copy(dst[:, tb_b * P:(tb_b + 1) * P], pst[:])
                else:
                    nc.vector.tensor_scalar(
                        dst[:, tb_b * P:(tb_b + 1) * P], pst[:],
                        g2[:], b2[:], op0=OP.mult, op1=OP.add)

        # ---- phase C: scores (row-tiled) -> exp -> ctx, then normalize ----
        def scores(b, qg, kb, pss):
            for h in range(HC):
                nc.tensor.matmul(
                    pss[:, h * QG:(h + 1) * QG],
                    kt_sb[b][h * HD:(h + 1) * HD, kb * P:(kb + 1) * P],
                    qt_sb[b][h * HD:(h + 1) * HD, qg * QG:(qg + 1) * QG],
                    start=True, stop=True)

        def c_qg(b, qg, fillers=()):
            fillers = list(fillers)
            ctx_ps = [pools["ps_ctx"].tile([HD + 1, QG], F32, name="ctx",
                                           tag="ctx") for _ in range(HC)]
            pss_tiles = {}
            pss_tiles[0] = pools["ps_s"].tile([P, 2 * QG], F32, name="pss",
                                              tag="pss")
            scores(b, qg, 0, pss_tiles[0])
            nfill = len(fillers)
            fill_at = {(i + 1) * KB // (nfill + 1): i for i in range(nfill)}
            for kb in range(KB):
                if kb + 1 < KB:
                    pss_tiles[kb + 1] = pools["ps_s"].tile(
                        [P, 2 * QG], F32, name="pss", tag="pss")
                    scores(b, qg, kb + 1, pss_tiles[kb + 1])
                at = pools["at"].tile([P, 2 * QG], BF16, name="at", tag="at")
                nc.scalar.activation(at[:], pss_tiles.pop(kb)[:], AF.Exp,
                                     bias=zero_sb[:], scale=0.125)
                for h in range(HC):
                    nc.tensor.matmul(
                        ctx_ps[h][:],
                        vp_sb[b][:, kb, h * (HD + 1):(h + 1) * (HD + 1)],
                        at[:, h * QG:(h + 1) * QG],
                        start=(kb == 0), stop=(kb == KB - 1))
                if kb in fill_at:
                    fillers[fill_at[kb]]()
            for h in range(HC):
                den = pools["rb"].tile([1, QG], F32, name="den", tag="den")
                nc.vector.tensor_copy(den[:], ctx_ps[h][HD:HD + 1, :])
                rec = pools["rb"].tile([1, QG], F32, name="rec", tag="rec")
                nc.vector.reciprocal_approx_fast(out=rec[:], in_=den[:])
                rb = pools["rb"].tile([HD, QG], F32, name="rb", tag="rb")
                nc.gpsimd.partition_broadcast(rb[:], rec[:])
                nc.vector.tensor_tensor(
                    out=cstage[b][h * HD:(h + 1) * HD,
                                  qg * QG:(qg + 1) * QG],
                    in0=ctx_ps[h][0:HD, :], in1=rb[:], op=OP.mult)
            nc.sync.dma_start(bounce[b][qg][:],
                              cstage[b][:, qg * QG:(qg + 1) * QG])
            nc.gpsimd.collective_compute(
                "AllGather", OP.bypass,
                replica_groups=[list(range(n_cores))],
                ins=[bounce[b][qg][:].opt()],
                outs=[gathered[b][qg][:].opt()])

        # ---- phase D: output projection of one gathered chunk ----
        def d_drain(b, qg, half=None):
            cols = QG if half is None else HQG
            src = gathered[b][qg] if half is None else gathered_h[half]
            col0 = b * NSEQ + qg * QG + (0 if half is None else half * HQG)
            pso = pools["ps_misc"].tile([P, cols], F32, name="pso", tag="misc")
            for dt in range(n_cores):
                prt = pools["pr"].tile([P, cols], BF16, name="prt", tag="pr")
                nc.gpsimd.dma_start(prt[:], src[dt * P:(dt + 1) * P, :])
                nc.tensor.matmul(pso[:], wp_sb[dt][:], prt[:],
                                 start=(dt == 0), stop=(dt == n_cores - 1))
            osb = pools["osb"].tile([P, cols], F32, name="osb", tag="osb")
            nc.vector.tensor_scalar(osb[:], pso[:], bp_sb[:], None, op0=OP.add)
            nc.sync.dma_start(outT[:, col0:col0 + cols], osb[:])

        # ---- schedule ----
        # Batch-0 first half evicts on the idle scalar engine; C(0) is issued
        # as soon as 4 token blocks are normalized, with the rest of B(0),
        # then all of A(1)/B(1), woven in as PE filler. Output drains only
        # cover gathers that are >= 2 chunks old so they never stall the PE
        # stream on an in-flight collective.
        for tb_b in range(NTB_B):
            if tb_b % 4 == 0 and tb_b // 4 + 2 < 4:
                load_xt_chunk(tb_b // 4 + 2)
            a_block(0, tb_b, scalar_evict=(tb_b < 8))
            if tb_b == NTB_B // 2 - 1:
                stats(0, 0)
        stats(0, 1)
        for tb_b in range(NTB_B):
            b_block(0, tb_b, scalar_evict=True)
        load_xt_chunk(4)
        load_xt_chunk(5)
        for dt in range(ND):
            nc.sync.dma_start(wp_sb[dt][:], wpT[dt * P:(dt + 1) * P, :])

        def a1_filler(tb_b):
            def f():
                if tb_b % 4 == 0 and tb_b // 4 + 6 < T // QG:
                    load_xt_chunk(tb_b // 4 + 6)
                a_block(1, tb_b)
                if tb_b == NTB_B // 2 - 1:
                    stats(1, 0)
                elif tb_b == NTB_B - 1:
                    stats(1, 1)
            return f

        c_qg(0, 0, fillers=[a1_filler(j) for j in range(4)])
        c_qg(0, 1, fillers=[a1_filler(4 + j) for j in range(4)])
        c_qg(0, 2, fillers=[a1_filler(8 + j) for j in range(4)])
        c_qg(0, 3, fillers=[a1_filler(12 + j) for j in range(4)])
        for tb_b in range(NTB_B):
            b_block(1, tb_b)

        c_qg(1, 0)
        c_qg(1, 1, fillers=[lambda: d_drain(0, 0)])
        c_qg(1, 2, fillers=[lambda: d_drain(0, 1), lambda: d_drain(1, 0)])
        c_qg(1, 3, fillers=[lambda: d_drain(0, 2), lambda: d_drain(0, 3),
                            lambda: d_drain(1, 1)])
        d_drain(1, 2)
        d_drain(1, 3)

    nc.compile()
    return nc


def prep_inputs(inputs):
    """Host-side prep: slice/transpose/cast per core. Returns (in_maps, trivial_gb)."""
    import ml_dtypes
    bf16 = ml_dtypes.bfloat16

    q = np.asarray(inputs["query"], np.float32)
    Wq, Wk, Wv, Wp = (np.asarray(inputs[k], np.float32)
                      for k in ("Wq", "Wk", "Wv", "Wp"))
    bq, bk, bv, bpv = (np.asarray(inputs[k], np.float32)
                       for k in ("bq", "bk", "bv", "bp"))
    qg, qb, kg, kb = (np.asarray(inputs[k], np.float32)
                      for k in ("q_gamma", "q_beta", "k_gamma", "k_beta"))

    trivial_gb = bool(
        np.all(qg == 1.0) and np.all(kg == 1.0)
        and np.all(qb == 0.0) and np.all(kb == 0.0))

    xT = np.ascontiguousarray(q.reshape(T, D).T).astype(bf16)
    identity = np.eye(P, dtype=bf16)
    in_maps = []
    for c in range(N_CORES):
        sl = slice(c * P, (c + 1) * P)
        wq_c, wk_c, wv_c = Wq[sl].T, Wk[sl].T, Wv[sl].T  # [1024, 128] each
        mean_cols = np.stack([
            wq_c[:, 0:HD].mean(axis=1), wq_c[:, HD:2 * HD].mean(axis=1),
            wk_c[:, 0:HD].mean(axis=1), wk_c[:, HD:2 * HD].mean(axis=1),
        ], axis=1)                                        # [1024, 4]
        wqkvT = np.concatenate([wq_c, wk_c, wv_c, mean_cols],
                               axis=1).astype(bf16)       # [1024, 388]
        bq_c, bk_c, bv_c = bq[sl], bk[sl], bv[sl]
        bias_means = np.array([
            bq_c[0:HD].mean(), bq_c[HD:].mean(),
            bk_c[0:HD].mean(), bk_c[HD:].mean()], np.float32)
        bqkv = np.concatenate([bq_c, bk_c, bv_c, bias_means])[None, :].astype(bf16)
        in_maps.append({
            "xT": xT,
            "wqkvT": np.ascontiguousarray(wqkvT),
            "bqkv": np.ascontiguousarray(bqkv),
            "wpT": np.ascontiguousarray(Wp[sl].T).astype(bf16),
            "bp": np.ascontiguousarray(bpv[sl].reshape(P, 1)),
            "qg2": np.tile(qg, HC).reshape(P, 1).astype(np.float32),
            "qb2": np.tile(qb, HC).reshape(P, 1).astype(np.float32),
            "kg2": np.tile(kg, HC).reshape(P, 1).astype(np.float32),
            "kb2": np.tile(kb, HC).reshape(P, 1).astype(np.float32),
            "ident": identity,
        })
    return in_maps, trivial_gb


def assemble_output(results):
    outT = np.concatenate([np.asarray(r["outT"], np.float32) for r in results],
                          axis=0)           # [1024, 4096]
    return np.ascontiguousarray(outT.T).reshape(B, NSEQ, D)


_CACHE = {}


def kernel(**inputs):
    from concourse.bass_utils import run_bass_kernel_spmd

    in_maps, _trivial = prep_inputs(inputs)
    key = ("nc", _trivial)
    if key not in _CACHE:
        _CACHE[key] = build(trivial_gb=_trivial)
    nc = _CACHE[key]
    res = run_bass_kernel_spmd(nc, in_maps, core_ids=list(range(N_CORES)))
    return assemble_output(res.results)


# revision 37
# speedup vs baseline: 1.2424x; 1.0371x over previous
"""Distributed QK-norm multi-head attention on 8 Trainium2 NeuronCores.

Strategy: tensor-parallel on heads (2 heads/core) through QKV projection and
attention; chunked AllGather of the head-major context (partition-axis
concat); column-parallel output projection. All operands are pre-transposed
and cast to bf16 on host so every matmul is in PE-native layout; f32
accumulation; softmax denominators via a ones-augmented V matmul.

v2 schedule: scalar engine runs ONLY softmax exp (one activation table load);
LN stats via Newton-rsqrt on the vector engine; PSUM evictions on
vector; row-tiled concurrent score matmuls (2 heads in disjoint PE row
groups); batch-1 QKV projection issued after batch-0 attention so it fills
PE gaps under the exp-bound softmax phase; per-chunk AllGather + output
projection drained inside the attention loops.

kernel(**inputs) takes the full unsharded inputs and returns the full
[2, 2048, 1024] float32 output.
"""

from contextlib import ExitStack

import numpy as np

import concourse.bass as bass
import concourse.bacc as bacc
import concourse.tile as tile
import concourse.mybir as mybir

F32 = mybir.dt.float32
BF16 = mybir.dt.bfloat16
I32 = mybir.dt.int32
AF = mybir.ActivationFunctionType
OP = mybir.AluOpType

N_CORES = 8
B, NSEQ, D = 2, 2048, 1024
H, HD = 16, 64
HC = H // N_CORES          # heads per core = 2
T = B * NSEQ               # 4096 tokens
P = 128
NTB = T // P               # 32 token blocks
NTB_B = NSEQ // P          # 16 per batch half
ND = D // P                # 8 contraction tiles
KB = NSEQ // P             # 16 key blocks
QG = 512                   # q-group (moving free dim)
NQG = NSEQ // QG           # 4 q groups
EPS = 1e-5
W = 3 * P + 4              # 388: qkv outputs + 4 mean columns
MAGIC = 0x5F3759DF         # rsqrt bit-trick seed


def build(n_cores: int = N_CORES, trivial_gb: bool = True):
    nc = bacc.Bacc("TRN2", target_bir_lowering=False, debug=False,
                   num_devices=n_cores)

    xT = nc.dram_tensor("xT", [D, T], BF16, kind="ExternalInput")
    wqkvT = nc.dram_tensor("wqkvT", [D, W], BF16, kind="ExternalInput")
    bqkv = nc.dram_tensor("bqkv", [1, W], BF16, kind="ExternalInput")
    wpT = nc.dram_tensor("wpT", [D, P], BF16, kind="ExternalInput")
    bp = nc.dram_tensor("bp", [P, 1], F32, kind="ExternalInput")
    qg2 = nc.dram_tensor("qg2", [P, 1], F32, kind="ExternalInput")
    qb2 = nc.dram_tensor("qb2", [P, 1], F32, kind="ExternalInput")
    kg2 = nc.dram_tensor("kg2", [P, 1], F32, kind="ExternalInput")
    kb2 = nc.dram_tensor("kb2", [P, 1], F32, kind="ExternalInput")
    ident = nc.dram_tensor("ident", [P, P], BF16, kind="ExternalInput")
    outT = nc.dram_tensor("outT", [P, T], F32, kind="ExternalOutput")

    with tile.TileContext(nc) as tc, ExitStack() as ctx:
        pools = {}
        for name, bufs, space in [
            ("xt", 1, "SBUF"), ("wq", 1, "SBUF"), ("wp", 1, "SBUF"),
            ("const", 1, "SBUF"), ("qkt", 1, "SBUF"), ("vp", 1, "SBUF"),
            ("raw", 1, "SBUF"), ("stat", 1, "SBUF"), ("sq", 2, "SBUF"),
            ("tok", 4, "SBUF"), ("at", 3, "SBUF"), ("rb", 3, "SBUF"),
            ("cstage", 1, "SBUF"), ("pr", 6, "SBUF"),
            ("osb", 2, "SBUF"), ("dram", 1, "DRAM"),
            ("ps_s", 2, "PSUM"), ("ps_ctx", 2, "PSUM"), ("ps_misc", 2, "PSUM"),
        ]:
            pools[name] = ctx.enter_context(
                tc.tile_pool(name=name, bufs=bufs, space=space))

        # ---- constants first: tiny DMAs must not queue behind bulk loads ----
        cp = pools["const"]
        bqkv_sb = cp.tile([1, W], BF16, name="bqkv_sb")
        nc.sync.dma_start(bqkv_sb[:], bqkv[:])
        bp_sb = cp.tile([P, 1], F32, name="bp_sb")
        nc.sync.dma_start(bp_sb[:], bp[:])
        gb_sb = {}
        for nm, src in (("qg2", qg2), ("qb2", qb2), ("kg2", kg2), ("kb2", kb2)):
            t_ = cp.tile([P, 1], F32, name=f"{nm}_sb")
            nc.sync.dma_start(t_[:], src[:])
            gb_sb[nm] = t_
        ident_sb = cp.tile([P, P], BF16, name="ident_sb")
        nc.sync.dma_start(ident_sb[:], ident[:])
        ones_sb = cp.tile([1, P], BF16, name="ones_sb")
        nc.vector.memset(ones_sb[:], 1.0)
        zero_sb = cp.tile([P, 1], F32, name="zero_sb")
        nc.vector.memset(zero_sb[:], 0.0)

        # ---- weights + x: interleave so chunk 0 + wq land together ----
        wq_sb = [pools["wq"].tile([P, W], BF16, name=f"wq{dt}")
                 for dt in range(ND)]
        wp_sb = [pools["wp"].tile([P, P], BF16, name=f"wp{dt}")
                 for dt in range(ND)]
        xt_sb = [pools["xt"].tile([P, T], BF16, name=f"xt{dt}")
                 for dt in range(ND)]

        def load_xt_cols(c0, c1):
            for dt in range(ND):
                nc.sync.dma_start(
                    xt_sb[dt][:, c0 * P:c1 * P],
                    xT[dt * P:(dt + 1) * P, c0 * P:c1 * P])

        def load_xt_chunk(ch):
            load_xt_cols(ch * (QG // P), (ch + 1) * (QG // P))

        for dt in range(ND):
            nc.sync.dma_start(wq_sb[dt][:], wqkvT[dt * P:(dt + 1) * P, :])
            nc.sync.dma_start(xt_sb[dt][:, 0:QG], xT[dt * P:(dt + 1) * P, 0:QG])
        load_xt_chunk(1)

        # warm the exp activation table before phase C needs it
        wex_in = cp.tile([1, 8], F32, name="wex_in")
        nc.vector.memset(wex_in[:], 0.0)
        wex_out = cp.tile([1, 8], BF16, name="wex_out")
        nc.scalar.activation(wex_out[:], wex_in[:], AF.Exp,
                             bias=zero_sb[0:1, :], scale=1.0)

        # persistent tensors
        qt_sb = [pools["qkt"].tile([P, NSEQ], BF16, name=f"qt{b}") for b in range(B)]
        kt_sb = [pools["qkt"].tile([P, NSEQ], BF16, name=f"kt{b}") for b in range(B)]
        vp_sb = [pools["vp"].tile([P, KB, HC * (HD + 1)], BF16, name=f"vp{b}")
                 for b in range(B)]
        for b in range(B):
            for h in range(HC):
                nc.vector.memset(
                    vp_sb[b][:, :, h * (HD + 1) + HD:h * (HD + 1) + HD + 1], 1.0)
        qkraw = [pools["raw"].tile([P, NTB_B, 2 * P], BF16, name=f"qkraw{b}")
                 for b in range(B)]
        svar = [pools["stat"].tile([P, 4 * NTB_B], F32, name=f"svar{b}")
                for b in range(B)]
        smu = [pools["stat"].tile([P, 4 * NTB_B], F32, name=f"smu{b}")
               for b in range(B)]
        rstd_all = [pools["stat"].tile([P, 4 * NTB_B], F32, name=f"rstd{b}")
                    for b in range(B)]
        cstage = [pools["cstage"].tile([P, NSEQ], BF16, name=f"cstage{b}")
                  for b in range(B)]

        warm_in = pools["dram"].tile([P, 4], BF16, name="warm_in")
        warm_out = pools["dram"].tile([P * n_cores, 4], BF16, name="warm_out",
                                      addr_space="Shared")
        warm_sb = cp.tile([P, 4], BF16, name="warm_sb")
        nc.vector.memset(warm_sb[:], 0.0)
        nc.sync.dma_start(warm_in[:], warm_sb[:])
        nc.gpsimd.collective_compute(
            "AllGather", OP.bypass, replica_groups=[list(range(n_cores))],
            ins=[warm_in[:].opt()], outs=[warm_out[:].opt()])

        bounce = [[pools["dram"].tile([P, QG], BF16, name=f"bounce{b}_{qg}")
                   for qg in range(NQG)] for b in range(B)]
        gathered = [[pools["dram"].tile([P * n_cores, QG], BF16,
                                        name=f"gath{b}_{qg}",
                                        addr_space="Shared")
                     for qg in range(NQG)] for b in range(B)]
        # final chunk is gathered in two halves to shorten the drain tail
        HQG = QG // 2
        bounce_h = [pools["dram"].tile([P, HQG], BF16, name=f"bounce_h{i}")
                    for i in range(2)]
        gathered_h = [pools["dram"].tile([P * n_cores, HQG], BF16,
                                         name=f"gath_h{i}",
                                         addr_space="Shared")
                      for i in range(2)]

        # ---- phase A: QKV projection for one token block ----
        # Batch 0 evicts on the (then idle) scalar engine; batch 1 runs under
        # phase C(0) where scalar is exp-saturated, so it evicts on vector.
        def a_block(b, tb_b, scalar_evict=False):
            tb = b * NTB_B + tb_b
            psp = pools["ps_misc"].tile([P, W], F32, name="psp", tag="misc")
            for dt in range(ND):
                nc.tensor.matmul(psp[:], xt_sb[dt][:, tb * P:(tb + 1) * P],
                                 wq_sb[dt][:], start=(dt == 0), stop=False)
            nc.tensor.matmul(psp[:], ones_sb[:], bqkv_sb[:],
                             start=False, stop=True)
            if scalar_evict:
                # scalar is idle pre-softmax; split eviction across engines
                nc.scalar.copy(qkraw[b][:, tb_b, :], psp[:, 0:2 * P])
                nc.scalar.copy(
                    vp_sb[b][:, tb_b, :].rearrange("p (h w) -> p h w",
                                                   h=HC)[:, :, 0:HD],
                    psp[:, 2 * P:3 * P].rearrange("p (h w) -> p h w", h=HC))
                nc.vector.tensor_copy(smu[b][:, 4 * tb_b:4 * tb_b + 4],
                                      psp[:, 3 * P:W])
            else:
                nc.vector.tensor_copy(qkraw[b][:, tb_b, :], psp[:, 0:2 * P])
                nc.vector.tensor_copy(
                    vp_sb[b][:, tb_b, :].rearrange("p (h w) -> p h w",
                                                   h=HC)[:, :, 0:HD],
                    psp[:, 2 * P:3 * P].rearrange("p (h w) -> p h w", h=HC))
                nc.vector.tensor_copy(smu[b][:, 4 * tb_b:4 * tb_b + 4],
                                      psp[:, 3 * P:W])
            sq = pools["sq"].tile([P, 2 * P], F32, name="sq", tag="sq")
            nc.vector.tensor_tensor(out=sq[:], in0=qkraw[b][:, tb_b, :],
                                    in1=qkraw[b][:, tb_b, :], op=OP.mult)
            nc.vector.tensor_reduce(
                svar[b][:, 4 * tb_b:4 * tb_b + 4],
                sq[:].rearrange("p (g w) -> p g w", g=4),
                axis=mybir.AxisListType.X, op=OP.add)

        # ---- stats: Newton-rsqrt of var+eps on the vector engine ----
        def stats(b, half):
            lo, hi = 32 * half, 32 * (half + 1)
            n = hi - lo
            musq = pools["sq"].tile([P, n], F32, name="musq", tag="musq")
            nc.vector.tensor_tensor(out=musq[:], in0=smu[b][:, lo:hi],
                                    in1=smu[b][:, lo:hi], op=OP.mult)
            varb = pools["sq"].tile([P, n], F32, name="varb", tag="varb")
            nc.vector.scalar_tensor_tensor(
                out=varb[:], in0=svar[b][:, lo:hi], scalar=1.0 / HD,
                in1=musq[:], op0=OP.mult, op1=OP.subtract)
            nc.vector.tensor_scalar(varb[:], varb[:], EPS, None, op0=OP.add)
            ib = pools["sq"].tile([P, n], I32, name="ib", tag="ib")
            nc.vector.tensor_scalar(ib[:], varb[:].bitcast(I32), 1, None,
                                    op0=OP.logical_shift_right)
            nc.vector.tensor_scalar(ib[:], ib[:], -1, None, op0=OP.bitwise_xor)
            nc.vector.tensor_scalar(ib[:], ib[:], MAGIC + 1, None, op0=OP.add)
            y = pools["sq"].tile([P, n], F32, name="y", tag="y")
            nc.vector.tensor_copy(y[:], ib[:].bitcast(F32))
            t_ = pools["sq"].tile([P, n], F32, name="t", tag="t")
            for it in range(2):
                nc.vector.tensor_tensor(out=t_[:], in0=y[:], in1=y[:],
                                        op=OP.mult)
                nc.vector.tensor_tensor(out=t_[:], in0=t_[:], in1=varb[:],
                                        op=OP.mult)
                nc.vector.tensor_scalar(t_[:], t_[:], -0.5, 1.5,
                                        op0=OP.mult, op1=OP.add)
                dst = rstd_all[b][:, lo:hi] if it == 1 else y[:]
                nc.vector.tensor_tensor(out=dst, in0=y[:], in1=t_[:],
                                        op=OP.mult)

        # ---- phase B: per-head QK layernorm + transpose to [hd, tokens] ----
        def b_block(b, tb_b, scalar_evict=False):
            for qk, (g2, b2, dst) in enumerate((
                    (gb_sb["qg2"], gb_sb["qb2"], qt_sb[b]),
                    (gb_sb["kg2"], gb_sb["kb2"], kt_sb[b]))):
                tokt = pools["tok"].tile([P, P], BF16, name="tokt", tag="tok")
                for h in range(HC):
                    i = 4 * tb_b + 2 * qk + h
                    nc.vector.tensor_scalar(
                        tokt[:, h * HD:(h + 1) * HD],
                        qkraw[b][:, tb_b,
                                 qk * P + h * HD:qk * P + (h + 1) * HD],
                        smu[b][:, i:i + 1], rstd_all[b][:, i:i + 1],
                        op0=OP.subtract, op1=OP.mult)
                pst = pools["ps_misc"].tile([P, P], BF16, name="pst",
                                            tag="misc")
                nc.tensor.transpose(pst[:], tokt[:], ident_sb[:])
                if scalar_evict and trivial_gb:
                    nc.scalar.copy(dst[:, tb_b * P:(tb_b + 1) * P], pst[:])
                else:
                    nc.vector.tensor_scalar(
                        dst[:, tb_b * P:(tb_b + 1) * P], pst[:],
                        g2[:], b2[:], op0=OP.mult, op1=OP.add)

        # ---- phase C: scores (row-tiled) -> exp -> ctx, then normalize ----
        def scores(b, qg, kb, pss):
            for h in range(HC):
                nc.tensor.matmul(
                    pss[:, h * QG:(h + 1) * QG],
                    kt_sb[b][h * HD:(h + 1) * HD, kb * P:(kb + 1) * P],
                    qt_sb[b][h * HD:(h + 1) * HD, qg * QG:(qg + 1) * QG],
                    start=True, stop=True)

        def c_qg(b, qg, fillers=(), pre_kb=None):
            fillers = list(fillers)
            pre_kb = pre_kb or {}
            ctx_ps = [pools["ps_ctx"].tile([HD + 1, QG], F32, name="ctx",
                                           tag="ctx") for _ in range(HC)]
            pss_tiles = {}
            pss_tiles[0] = pools["ps_s"].tile([P, 2 * QG], F32, name="pss",
                                              tag="pss")
            scores(b, qg, 0, pss_tiles[0])
            nfill = len(fillers)
            fill_at = {(i + 1) * KB // (nfill + 1): i for i in range(nfill)}
            for kb in range(KB):
                for f in pre_kb.get(kb, ()):
                    f()
                if kb + 1 < KB:
                    pss_tiles[kb + 1] = pools["ps_s"].tile(
                        [P, 2 * QG], F32, name="pss", tag="pss")
                    scores(b, qg, kb + 1, pss_tiles[kb + 1])
                at = pools["at"].tile([P, 2 * QG], BF16, name="at", tag="at")
                nc.scalar.activation(at[:], pss_tiles.pop(kb)[:], AF.Exp,
                                     bias=zero_sb[:], scale=0.125)
                for h in range(HC):
                    nc.tensor.matmul(
                        ctx_ps[h][:],
                        vp_sb[b][:, kb, h * (HD + 1):(h + 1) * (HD + 1)],
                        at[:, h * QG:(h + 1) * QG],
                        start=(kb == 0), stop=(kb == KB - 1))
                if kb in fill_at:
                    fillers[fill_at[kb]]()
            for h in range(HC):
                den = pools["rb"].tile([1, QG], F32, name="den", tag="den")
                nc.vector.tensor_copy(den[:], ctx_ps[h][HD:HD + 1, :])
                rec = pools["rb"].tile([1, QG], F32, name="rec", tag="rec")
                nc.vector.reciprocal_approx_fast(out=rec[:], in_=den[:])
                rb = pools["rb"].tile([HD, QG], F32, name="rb", tag="rb")
                nc.gpsimd.partition_broadcast(rb[:], rec[:])
                nc.vector.tensor_tensor(
                    out=cstage[b][h * HD:(h + 1) * HD,
                                  qg * QG:(qg + 1) * QG],
                    in0=ctx_ps[h][0:HD, :], in1=rb[:], op=OP.mult)
            nc.sync.dma_start(bounce[b][qg][:],
                              cstage[b][:, qg * QG:(qg + 1) * QG])
            nc.gpsimd.collective_compute(
                "AllGather", OP.bypass,
                replica_groups=[list(range(n_cores))],
                ins=[bounce[b][qg][:].opt()],
                outs=[gathered[b][qg][:].opt()])

        # ---- phase D: output projection of one gathered chunk ----
        def d_drain(b, qg, half=None):
            cols = QG if half is None else HQG
            src = gathered[b][qg] if half is None else gathered_h[half]
            col0 = b * NSEQ + qg * QG + (0 if half is None else half * HQG)
            pso = pools["ps_misc"].tile([P, cols], F32, name="pso", tag="misc")
            for dt in range(n_cores):
                prt = pools["pr"].tile([P, cols], BF16, name="prt", tag="pr")
                nc.sync.dma_start(prt[:], src[dt * P:(dt + 1) * P, :])
                nc.tensor.matmul(pso[:], wp_sb[dt][:], prt[:],
                                 start=(dt == 0), stop=(dt == n_cores - 1))
            osb = pools["osb"].tile([P, cols], F32, name="osb", tag="osb")
            nc.vector.tensor_scalar(osb[:], pso[:], bp_sb[:], None, op0=OP.add)
            nc.sync.dma_start(outT[:, col0:col0 + cols], osb[:])

        # ---- schedule ----
        # Batch-0 first half evicts on the idle scalar engine; C(0) is issued
        # as soon as 4 token blocks are normalized, with the rest of B(0),
        # then all of A(1)/B(1), woven in as PE filler. Output drains only
        # cover gathers that are >= 2 chunks old so they never stall the PE
        # stream on an in-flight collective.
        load_xt_chunk(2)
        load_xt_chunk(3)
        for tb_b in range(NTB_B):
            a_block(0, tb_b, scalar_evict=(tb_b < 8))
            if tb_b == NTB_B // 2 - 1:
                stats(0, 0)
        stats(0, 1)
        for tb_b in range(6):
            b_block(0, tb_b, scalar_evict=True)
        load_xt_chunk(4)
        load_xt_chunk(5)
        for dt in range(ND):
            nc.sync.dma_start(wp_sb[dt][:], wpT[dt * P:(dt + 1) * P, :])

        def a1_filler(tb_b):
            def f():
                if tb_b % 4 == 0 and tb_b // 4 + 6 < T // QG:
                    load_xt_chunk(tb_b // 4 + 6)
                a_block(1, tb_b)
                if tb_b == NTB_B // 2 - 1:
                    stats(1, 0)
                elif tb_b == NTB_B - 1:
                    stats(1, 1)
            return f

        c_qg(0, 0,
             pre_kb={j: [(lambda tb: (lambda: b_block(0, tb,
                                                      scalar_evict=True)))(j + 6)]
                     for j in range(10)})
        c_qg(0, 1, fillers=[a1_filler(j) for j in range(5)])
        c_qg(0, 2, fillers=[a1_filler(5 + j) for j in range(6)])
        c_qg(0, 3, fillers=[a1_filler(11 + j) for j in range(5)])
        for tb_b in range(NTB_B):
            b_block(1, tb_b)

        c_qg(1, 0)
        c_qg(1, 1, fillers=[lambda: d_drain(0, 0)])
        c_qg(1, 2, fillers=[lambda: d_drain(0, 1), lambda: d_drain(1, 0)])
        c_qg(1, 3, fillers=[lambda: d_drain(0, 2), lambda: d_drain(0, 3),
                            lambda: d_drain(1, 1), lambda: d_drain(1, 2)])
        d_drain(1, 3)

    nc.compile()
    return nc


def prep_inputs(inputs):
    """Host-side prep: slice/transpose/cast per core. Returns (in_maps, trivial_gb)."""
    import ml_dtypes
    bf16 = ml_dtypes.bfloat16

    q = np.asarray(inputs["query"], np.float32)
    Wq, Wk, Wv, Wp = (np.asarray(inputs[k], np.float32)
                      for k in ("Wq", "Wk", "Wv", "Wp"))
    bq, bk, bv, bpv = (np.asarray(inputs[k], np.float32)
                       for k in ("bq", "bk", "bv", "bp"))
    qg, qb, kg, kb = (np.asarray(inputs[k], np.float32)
                      for k in ("q_gamma", "q_beta", "k_gamma", "k_beta"))

    trivial_gb = bool(
        np.all(qg == 1.0) and np.all(kg == 1.0)
        and np.all(qb == 0.0) and np.all(kb == 0.0))

    xT = np.ascontiguousarray(q.reshape(T, D).T).astype(bf16)
    identity = np.eye(P, dtype=bf16)
    in_maps = []
    for c in range(N_CORES):
        sl = slice(c * P, (c + 1) * P)
        wq_c, wk_c, wv_c = Wq[sl].T, Wk[sl].T, Wv[sl].T  # [1024, 128] each
        mean_cols = np.stack([
            wq_c[:, 0:HD].mean(axis=1), wq_c[:, HD:2 * HD].mean(axis=1),
            wk_c[:, 0:HD].mean(axis=1), wk_c[:, HD:2 * HD].mean(axis=1),
        ], axis=1)                                        # [1024, 4]
        wqkvT = np.concatenate([wq_c, wk_c, wv_c, mean_cols],
                               axis=1).astype(bf16)       # [1024, 388]
        bq_c, bk_c, bv_c = bq[sl], bk[sl], bv[sl]
        bias_means = np.array([
            bq_c[0:HD].mean(), bq_c[HD:].mean(),
            bk_c[0:HD].mean(), bk_c[HD:].mean()], np.float32)
        bqkv = np.concatenate([bq_c, bk_c, bv_c, bias_means])[None, :].astype(bf16)
        in_maps.append({
            "xT": xT,
            "wqkvT": np.ascontiguousarray(wqkvT),
            "bqkv": np.ascontiguousarray(bqkv),
            "wpT": np.ascontiguousarray(Wp[sl].T).astype(bf16),
            "bp": np.ascontiguousarray(bpv[sl].reshape(P, 1)),
            "qg2": np.tile(qg, HC).reshape(P, 1).astype(np.float32),
            "qb2": np.tile(qb, HC).reshape(P, 1).astype(np.float32),
            "kg2": np.tile(kg, HC).reshape(P, 1).astype(np.float32),
            "kb2": np.tile(kb, HC).reshape(P, 1).astype(np.float32),
            "ident": identity,
        })
    return in_maps, trivial_gb


def assemble_output(results):
    outT = np.concatenate([np.asarray(r["outT"], np.float32) for r in results],
                          axis=0)           # [1024, 4096]
    return np.ascontiguousarray(outT.T).reshape(B, NSEQ, D)


_CACHE = {}


def kernel(**inputs):
    from concourse.bass_utils import run_bass_kernel_spmd

    in_maps, _trivial = prep_inputs(inputs)
    key = ("nc", _trivial)
    if key not in _CACHE:
        _CACHE[key] = build(trivial_gb=_trivial)
    nc = _CACHE[key]
    res = run_bass_kernel_spmd(nc, in_maps, core_ids=list(range(N_CORES)))
    return assemble_output(res.results)
